# revision 6
# baseline (speedup 1.0000x reference)
"""GCN autoencoder (8x GCNConv) on 8 Trainium2 NeuronCores.

Sharding: nodes row-sharded across the 8 cores (6250 rows each). Per layer:
  1. local matmul  h'_c = dinv * (T_c @ W)           (TensorE, bf16)
  2. AllGather of the bf16 gather-table to every core's DRAM
  3. edge phase: batched dma_gather of this core's dst-edges' source rows,
     segment-sum via one-hot matmuls accumulated in PSUM, epilogue applies
     dinv / bias / relu.
Self-loops are materialized as explicit (i, i) edges. Edges whose src is
>= 32768 use a second gather stream rebased against the table's upper half
(dma_gather indices are int16).
"""

import sys

sys.path.insert(0, "/opt/trn_rl_repo")

import numpy as np
import ml_dtypes

N, E, F = 50000, 800000, 500
NCORES = 8
NPC = N // NCORES            # 6250 rows per core
NBLK = (NPC + 127) // 128    # 49 dst blocks per core
LAST_ROWS = NPC - (NBLK - 1) * 128  # 106
LO_BASE = 32768              # int16 index limit for dma_gather
GCH = 8                      # chunks (of 128 edges) per dma_gather batch
PAD_DSTREL = 200.0

BF16 = ml_dtypes.bfloat16

_CACHE = {}


# --------------------------------------------------------------------------
# host-side preprocessing
# --------------------------------------------------------------------------

def _build_host_data(x, src, dst):
    """Edge schedule + per-core input arrays. Pure numpy."""
    deg = np.bincount(dst, minlength=N).astype(np.float64) + 1.0
    dinv = (1.0 / np.sqrt(deg)).astype(np.float32)

    # append self edges
    s_all = np.concatenate([src.astype(np.int64), np.arange(N, dtype=np.int64)])
    d_all = np.concatenate([dst.astype(np.int64), np.arange(N, dtype=np.int64)])

    core = d_all // NPC
    lo_lists = [[None] * NBLK for _ in range(NCORES)]
    hi_lists = [[None] * NBLK for _ in range(NCORES)]
    rel_lo = [[None] * NBLK for _ in range(NCORES)]
    rel_hi = [[None] * NBLK for _ in range(NCORES)]
    for c in range(NCORES):
        m = core == c
        sc, dc = s_all[m], d_all[m] - c * NPC
        blk = dc // 128
        rel = dc % 128
        order = np.argsort(blk, kind="stable")
        sc, blk, rel = sc[order], blk[order], rel[order]
        bounds = np.searchsorted(blk, np.arange(NBLK + 1))
        for b in range(NBLK):
            sb = sc[bounds[b]:bounds[b + 1]]
            rb = rel[bounds[b]:bounds[b + 1]]
            lo = sb < LO_BASE
            lo_lists[c][b] = sb[lo]
            rel_lo[c][b] = rb[lo]
            hi_lists[c][b] = sb[~lo] - LO_BASE
            rel_hi[c][b] = rb[~lo]

    L = [max(int(np.ceil(len(lo_lists[c][b]) / 128)) for c in range(NCORES))
         for b in range(NBLK)]
    Hc = [max(int(np.ceil(len(hi_lists[c][b]) / 128)) for c in range(NCORES))
          for b in range(NBLK)]
    TL, TH = sum(L), sum(Hc)

    def pack_stream(counts, src_ll, rel_ll):
        tot = sum(counts)
        idx = np.zeros((NCORES, tot * 128), np.int64)
        rel = np.full((NCORES, tot * 128), PAD_DSTREL, np.float32)
        off = 0
        for b, nch in enumerate(counts):
            for c in range(NCORES):
                s, r = src_ll[c][b], rel_ll[c][b]
                idx[c, off:off + len(s)] = s
                rel[c, off:off + len(r)] = r
            off += nch * 128
        return idx, rel

    idx_lo, drel_lo = pack_stream(L, lo_lists, rel_lo)
    idx_hi, drel_hi = pack_stream(Hc, hi_lists, rel_hi)

    def idx_layout(idx):
        # [ncore, T*128] -> [ncore, 128, T*8] int16 (16-partition wrap,
        # replicated across the 8 q7 cores)
        nc_, tot = idx.shape
        a = idx.reshape(nc_, tot // 16, 16).transpose(0, 2, 1)  # [nc,16,tot/16]
        return np.ascontiguousarray(np.tile(a, (1, 8, 1))).astype(np.int16)

    def rel_layout(rel):
        nc_, tot = rel.shape
        return np.ascontiguousarray(
            rel.reshape(nc_, tot // 128, 128).transpose(0, 2, 1)
        ).astype(BF16)

    dinv_t = np.ones((NCORES, 128, NBLK), np.float32)
    for c in range(NCORES):
        dv = dinv[c * NPC:(c + 1) * NPC]
        pad = np.concatenate([dv, np.ones(NBLK * 128 - NPC, np.float32)])
        dinv_t[c] = pad.reshape(NBLK, 128).T

    xt = np.zeros((NCORES, 512, NPC), BF16)
    for c in range(NCORES):
        xt[c, :F, :] = x[c * NPC:(c + 1) * NPC].T.astype(BF16)

    return dict(
        dinv=dinv,
        L=L, H=Hc, TL=TL, TH=TH,
        idx_lo=idx_layout(idx_lo), idx_hi=idx_layout(idx_hi),
        drel_lo=rel_layout(drel_lo), drel_hi=rel_layout(drel_hi),
        dinv_t=dinv_t, xt=xt,
    )


def _weight_tiles(w, din_pad, dout_pad):
    """[din, dout] f32 -> [128, din_pad//128, dout_pad] bf16 host layout."""
    din, dout = w.shape
    wp = np.zeros((din_pad, dout_pad), np.float32)
    wp[:din, :dout] = w
    nk = din_pad // 128
    return np.ascontiguousarray(
        wp.reshape(nk, 128, dout_pad).transpose(1, 0, 2)
    ).astype(BF16)


def _bias_tile(b, dout_pad):
    bp = np.zeros(dout_pad, np.float32)
    bp[: len(b)] = b
    return np.tile(bp, (128, 1)).astype(np.float32)


# layer table: kind 'mm' = step-1 matmul then aggregate (post-agg)
#              'pre'    = aggregate table then matmul (pre-agg)
LAYERS = [
    dict(nm="enc1", din=512, dout=256, tbl=256, relu=True, out="enc_h1", kind="mm"),
    dict(nm="enc2", din=256, dout=256, tbl=256, relu=True, out="enc_h2", kind="mm"),
    dict(nm="enc3", din=256, dout=256, tbl=256, relu=True, out="enc_h3", kind="mm"),
    dict(nm="zen", din=256, dout=128, tbl=128, relu=False, out="z_en", kind="mm",
         make_zp=True, no_t=True),
    dict(nm="dec1", din=128, dout=256, tbl=128, relu=True, out=None, kind="pre"),
    dict(nm="dec2", din=256, dout=256, tbl=256, relu=True, out=None, kind="mm"),
    dict(nm="dec3", din=256, dout=256, tbl=256, relu=True, out=None, kind="mm"),
    dict(nm="xde", din=256, dout=500, tbl=256, relu=False, out="x_de", kind="pre",
         no_t=True),
]


# --------------------------------------------------------------------------
# bass graph
# --------------------------------------------------------------------------

def _build_graph(L, Hc, TL, TH):
    import contextlib
    import concourse.bacc as bacc
    import concourse.mybir as mybir
    import concourse.tile as tile

    fp32 = mybir.dt.float32
    bf16 = mybir.dt.bfloat16
    i16 = mybir.dt.int16
    AOT = mybir.AluOpType

    nc = bacc.Bacc("TRN2")

    # ---- dram parameters ----
    t_xt = nc.dram_tensor("xt", [512, NPC], bf16, kind="ExternalInput")
    t_il = nc.dram_tensor("il", [128, TL * 8], i16, kind="ExternalInput")
    t_ih = nc.dram_tensor("ih", [128, TH * 8], i16, kind="ExternalInput")
    t_dl = nc.dram_tensor("dl", [128, TL], bf16, kind="ExternalInput")
    t_dh = nc.dram_tensor("dh", [128, TH], bf16, kind="ExternalInput")
    t_dinv = nc.dram_tensor("dinv_t", [128, NBLK], fp32, kind="ExternalInput")
    t_iota = nc.dram_tensor("iota", [128, 128], bf16, kind="ExternalInput")
    t_ident = nc.dram_tensor("ident", [128, 128], bf16, kind="ExternalInput")
    t_w, t_b = {}, {}
    for ly in LAYERS:
        nk = ly["din"] // 128
        t_w[ly["nm"]] = nc.dram_tensor("w_" + ly["nm"], [128, nk, ly["dout"]],
                                       bf16, kind="ExternalInput")
        t_b[ly["nm"]] = nc.dram_tensor("b_" + ly["nm"], [128, ly["dout"]],
                                       fp32, kind="ExternalInput")
    t_out = {
        "enc_h1": nc.dram_tensor("enc_h1", [NPC, 256], fp32, kind="ExternalOutput"),
        "enc_h2": nc.dram_tensor("enc_h2", [NPC, 256], fp32, kind="ExternalOutput"),
        "enc_h3": nc.dram_tensor("enc_h3", [NPC, 256], fp32, kind="ExternalOutput"),
        "z_en": nc.dram_tensor("z_en", [NPC, 128], fp32, kind="ExternalOutput"),
        "x_de": nc.dram_tensor("x_de", [NPC, 500], fp32, kind="ExternalOutput"),
    }

    bounces, tbls = [], []
    for i, ly in enumerate(LAYERS):
        w = ly["tbl"]
        bounces.append(nc.dram_tensor(f"bnc{i}", [NPC, w], bf16, kind="Internal"))
        tbls.append(nc.dram_tensor(f"tbl{i}", [N, w], bf16, kind="Internal",
                                   addr_space="Shared"))

    lo_off = np.concatenate([[0], np.cumsum(L)]).astype(int)
    hi_off = np.concatenate([[0], np.cumsum(Hc)]).astype(int)

    with tile.TileContext(nc) as tc:
        ctx = contextlib.ExitStack()
        const = ctx.enter_context(tc.tile_pool(name="const", bufs=1))
        wpool = ctx.enter_context(tc.tile_pool(name="w", bufs=1))
        tpool = ctx.enter_context(tc.tile_pool(name="T", bufs=2))
        xtp = ctx.enter_context(tc.tile_pool(name="xt", bufs=4))
        mlo = ctx.enter_context(tc.tile_pool(name="mlo", bufs=2))
        mhi = ctx.enter_context(tc.tile_pool(name="mhi", bufs=2))
        ohp = ctx.enter_context(tc.tile_pool(name="oh", bufs=2))
        epp = ctx.enter_context(tc.tile_pool(name="ep", bufs=3))
        ttp = ctx.enter_context(tc.tile_pool(name="tt", bufs=4))
        psE = ctx.enter_context(tc.tile_pool(name="psE", bufs=2, space="PSUM"))
        psM = ctx.enter_context(tc.tile_pool(name="psM", bufs=2, space="PSUM"))
        ps2p = ctx.enter_context(tc.tile_pool(name="ps2p", bufs=2, space="PSUM"))
        psT = ctx.enter_context(tc.tile_pool(name="psT", bufs=2, space="PSUM"))

        # ---- constants into SBUF ----
        def load_const(t, shape, dt, tag):
            s = const.tile(shape, dt, tag=tag)
            nc.sync.dma_start(out=s[:], in_=t[:])
            return s

        il_sb = load_const(t_il, [128, TL * 8], i16, "c_il")
        ih_sb = load_const(t_ih, [128, TH * 8], i16, "c_ih")
        dl_sb = load_const(t_dl, [128, TL], bf16, "c_dl")
        dh_sb = load_const(t_dh, [128, TH], bf16, "c_dh")
        dinv_sb = load_const(t_dinv, [128, NBLK], fp32, "c_dinv")
        iota_sb = load_const(t_iota, [128, 128], bf16, "c_iota")
        ident_sb = load_const(t_ident, [128, 128], bf16, "c_ident")
        w_sb, b_sb = {}, {}
        for ly in LAYERS:
            nk = ly["din"] // 128
            w_sb[ly["nm"]] = load_const(t_w[ly["nm"]], [128, nk, ly["dout"]],
                                        bf16, "c_w_" + ly["nm"])
            b_sb[ly["nm"]] = load_const(t_b[ly["nm"]], [128, ly["dout"]],
                                        fp32, "c_b_" + ly["nm"])

        def rows(b):
            return 128 if b < NBLK - 1 else LAST_ROWS

        def transpose_128(src_ap, dst_bf16_ap):
            pt = psT.tile([128, 128], bf16, space="PSUM", tag="psT")
            nc.tensor.transpose(out=pt[:], in_=src_ap, identity=ident_sb[:])
            nc.vector.tensor_copy(out=dst_bf16_ap, in_=pt[:])

        def step1_matmul(ly, li, T_tiles):
            """h' = dinv*(T @ W) -> bounce[li].  T_tiles None => use xt input."""
            w = ly["tbl"]
            nk = ly["din"] // 128
            for b in range(NBLK):
                ps = psM.tile([128, w], fp32, space="PSUM", tag="psM")
                for k in range(nk):
                    if T_tiles is None:
                        xk = xtp.tile([128, 128], bf16, tag="xk")
                        nc.sync.dma_start(
                            out=xk[:, :rows(b)],
                            in_=t_xt[k * 128:(k + 1) * 128,
                                     b * 128:b * 128 + rows(b)],
                        )
                        lhsT = xk[:, :rows(b)]
                    else:
                        tt = ttp.tile([128, 128], bf16, tag="ttmm")
                        transpose_128(T_tiles[:, b, k * 128:(k + 1) * 128], tt[:])
                        lhsT = tt[:, :rows(b)]
                    nc.tensor.matmul(
                        out=ps[:rows(b), :], lhsT=lhsT,
                        rhs=w_sb[ly["nm"]][:, k, :],
                        start=(k == 0), stop=(k == nk - 1),
                    )
                hp = epp.tile([128, w], bf16, tag="hprime")
                nc.vector.tensor_scalar_mul(
                    out=hp[:rows(b), :], in0=ps[:rows(b), :],
                    scalar1=dinv_sb[:rows(b), b:b + 1],
                )
                nc.gpsimd.dma_start(
                    out=bounces[li][b * 128:b * 128 + rows(b), :],
                    in_=hp[:rows(b), :],
                )

        def scale_only(ly, li, T_tiles):
            w = ly["tbl"]
            for b in range(NBLK):
                hp = epp.tile([128, w], bf16, tag="hprime")
                nc.vector.tensor_scalar_mul(
                    out=hp[:rows(b), :], in0=T_tiles[:rows(b), b, :],
                    scalar1=dinv_sb[:rows(b), b:b + 1],
                )
                nc.gpsimd.dma_start(
                    out=bounces[li][b * 128:b * 128 + rows(b), :],
                    in_=hp[:rows(b), :],
                )

        def all_gather(li):
            nc.gpsimd.collective_compute(
                "AllGather", AOT.bypass,
                replica_groups=[list(range(NCORES))],
                ins=[bounces[li][:].opt()],
                outs=[tbls[li][:].opt()],
            )

        def edge_phase(ly, li):
            w = ly["tbl"]
            n_lo_batch = (TL + GCH - 1) // GCH
            n_hi_batch = (TH + GCH - 1) // GCH
            lo_tiles = [None] * n_lo_batch
            hi_tiles = [None] * n_hi_batch
            oh_lo = [None] * n_lo_batch
            oh_hi = [None] * n_hi_batch

            def fetch(bt, stream):
                if stream == "lo":
                    nchunk = min(GCH, TL - bt * GCH)
                    m = mlo.tile([128, nchunk, w], bf16, tag="mlo")
                    nc.gpsimd.dma_gather(
                        m[:], tbls[li][:LO_BASE, :],
                        il_sb[:, bt * GCH * 8:(bt * GCH + nchunk) * 8],
                        nchunk * 128, nchunk * 128, w,
                    )
                    lo_tiles[bt] = m
                    oh = ohp.tile([128, nchunk, 128], bf16, tag="oh")
                    nc.vector.tensor_tensor(
                        out=oh[:],
                        in0=dl_sb[:, bt * GCH:bt * GCH + nchunk, None]
                        .to_broadcast([128, nchunk, 128]),
                        in1=iota_sb[:, None, :].to_broadcast([128, nchunk, 128]),
                        op=AOT.is_equal,
                    )
                    oh_lo[bt] = oh
                else:
                    nchunk = min(GCH, TH - bt * GCH)
                    m = mhi.tile([128, nchunk, w], bf16, tag="mhi")
                    nc.gpsimd.dma_gather(
                        m[:], tbls[li][LO_BASE:, :],
                        ih_sb[:, bt * GCH * 8:(bt * GCH + nchunk) * 8],
                        nchunk * 128, nchunk * 128, w,
                    )
                    hi_tiles[bt] = m
                    oh = ohp.tile([128, nchunk, 128], bf16, tag="ohh")
                    nc.vector.tensor_tensor(
                        out=oh[:],
                        in0=dh_sb[:, bt * GCH:bt * GCH + nchunk, None]
                        .to_broadcast([128, nchunk, 128]),
                        in1=iota_sb[:, None, :].to_broadcast([128, nchunk, 128]),
                        op=AOT.is_equal,
                    )
                    oh_hi[bt] = oh

            lo_done = hi_done = 0
            for b in range(NBLK):
                ps = psE.tile([128, w], fp32, space="PSUM", tag="psE")
                nmm = L[b] + Hc[b]
                i = 0
                for ch in range(lo_off[b], lo_off[b] + L[b]):
                    bt, sl = ch // GCH, ch % GCH
                    while lo_done <= bt:
                        fetch(lo_done, "lo")
                        lo_done += 1
                    nc.tensor.matmul(
                        out=ps[:], lhsT=oh_lo[bt][:, sl, :],
                        rhs=lo_tiles[bt][:, sl, :],
                        start=(i == 0), stop=(i == nmm - 1),
                    )
                    i += 1
                for ch in range(hi_off[b], hi_off[b] + Hc[b]):
                    bt, sl = ch // GCH, ch % GCH
                    while hi_done <= bt:
                        fetch(hi_done, "hi")
                        hi_done += 1
                    nc.tensor.matmul(
                        out=ps[:], lhsT=oh_hi[bt][:, sl, :],
                        rhs=hi_tiles[bt][:, sl, :],
                        start=(i == 0), stop=(i == nmm - 1),
                    )
                    i += 1
                yield b, ps

        # ------------------------------------------------------------------
        T_cur = None
        for li, ly in enumerate(LAYERS):
            w = ly["tbl"]
            nm = ly["nm"]
            if ly["kind"] == "mm":
                step1_matmul(ly, li, T_cur if li else None)
            elif nm == "xde":
                scale_only(ly, li, T_cur)
            # (dec1's bounce was filled by zen's epilogue)
            all_gather(li)

            T_next = None
            if not ly.get("no_t"):
                T_next = tpool.tile([128, NBLK, ly["dout"]], bf16, tag="T")

            for b, ps in edge_phase(ly, li):
                r = rows(b)
                if ly["kind"] == "mm":
                    s = epp.tile([128, w], fp32, tag="s")
                    nc.vector.scalar_tensor_tensor(
                        out=s[:], in0=ps[:],
                        scalar=dinv_sb[:, b:b + 1], in1=b_sb[nm][:],
                        op0=AOT.mult, op1=AOT.add,
                    )
                    if ly["relu"]:
                        o = epp.tile([128, w], fp32, tag="o")
                        nc.vector.tensor_scalar_max(out=o[:], in0=s[:], scalar1=0.0)
                    else:
                        o = s
                    if ly["out"]:
                        nc.sync.dma_start(
                            out=t_out[ly["out"]][b * 128:b * 128 + r, :],
                            in_=o[:r, :],
                        )
                    if ly.get("make_zp"):
                        zp = epp.tile([128, w], bf16, tag="zp")
                        nc.vector.tensor_scalar_mul(
                            out=zp[:r, :], in0=o[:r, :],
                            scalar1=dinv_sb[:r, b:b + 1],
                        )
                        nc.gpsimd.dma_start(
                            out=bounces[li + 1][b * 128:b * 128 + r, :],
                            in_=zp[:r, :],
                        )
                    if T_next is not None:
                        nc.vector.tensor_copy(out=T_next[:, b, :], in_=o[:])
                else:
                    # pre-agg: A = ps*dinv -> transpose -> @W +b (+relu)
                    a = epp.tile([128, w], bf16, tag="a")
                    nc.vector.tensor_scalar_mul(
                        out=a[:], in0=ps[:], scalar1=dinv_sb[:, b:b + 1],
                    )
                    nk2 = w // 128
                    ps2 = ps2p.tile([128, ly["dout"]], fp32, space="PSUM",
                                    tag="ps2")
                    for k in range(nk2):
                        at = ttp.tile([128, 128], bf16, tag="at")
                        transpose_128(a[:, k * 128:(k + 1) * 128], at[:])
                        nc.tensor.matmul(
                            out=ps2[:], lhsT=at[:],
                            rhs=w_sb[nm][:, k, :],
                            start=(k == 0), stop=(k == nk2 - 1),
                        )
                    s = epp.tile([128, ly["dout"]], fp32, tag="s2")
                    nc.vector.scalar_tensor_tensor(
                        out=s[:], in0=ps2[:], scalar=1.0,
                        in1=b_sb[nm][:], op0=AOT.mult, op1=AOT.add,
                    )
                    if ly["relu"]:
                        o = epp.tile([128, ly["dout"]], fp32, tag="o2")
                        nc.vector.tensor_scalar_max(
                            out=o[:], in0=s[:], scalar1=0.0)
                    else:
                        o = s
                    if ly["out"]:
                        nc.sync.dma_start(
                            out=t_out[ly["out"]][b * 128:b * 128 + r, :],
                            in_=o[:r, :],
                        )
                    if T_next is not None:
                        nc.vector.tensor_copy(out=T_next[:, b, :], in_=o[:])
            if T_next is not None:
                T_cur = T_next
        ctx.close()

    nc.compile()
    return nc


# --------------------------------------------------------------------------
# public entry
# --------------------------------------------------------------------------

def _prepare(inputs):
    x = np.asarray(inputs["x"], np.float32)
    src = np.asarray(inputs["src"])
    dst = np.asarray(inputs["dst"])

    host = _build_host_data(x, src, dst)
    key = (hash(src.tobytes()) ^ hash(dst.tobytes()),
           tuple(host["L"]), tuple(host["H"]))
    if key in _CACHE:
        nc = _CACHE[key]
    else:
        nc = _build_graph(host["L"], host["H"], host["TL"], host["TH"])
        _CACHE[key] = nc

    iota = np.tile(np.arange(128, dtype=np.float32), (128, 1)).astype(BF16)
    ident = np.eye(128, dtype=np.float32).astype(BF16)
    shared = {"iota": iota, "ident": ident}
    for ly in LAYERS:
        shared["w_" + ly["nm"]] = _weight_tiles(
            np.asarray(inputs["W_" + ly["nm"]], np.float32), ly["din"], ly["dout"])
        shared["b_" + ly["nm"]] = _bias_tile(
            np.asarray(inputs["b_" + ly["nm"]], np.float32), ly["dout"])

    in_maps = []
    for c in range(NCORES):
        m = dict(shared)
        m["xt"] = host["xt"][c]
        m["il"] = host["idx_lo"][c]
        m["ih"] = host["idx_hi"][c]
        m["dl"] = host["drel_lo"][c]
        m["dh"] = host["drel_hi"][c]
        m["dinv_t"] = host["dinv_t"][c]
        in_maps.append(m)
    return nc, in_maps


def _assemble(results):
    def cat(name, wcut):
        return np.concatenate(
            [results[c][name][:, :wcut] for c in range(NCORES)], axis=0)

    x_de = cat("x_de", 500)
    enc_h1 = cat("enc_h1", 256)
    enc_h2 = cat("enc_h2", 256)
    enc_h3 = cat("enc_h3", 256)
    z_en = cat("z_en", 10)
    return x_de, (enc_h1, enc_h2, enc_h3), z_en


def kernel(**inputs):
    from concourse.bass_utils import run_bass_kernel_spmd

    nc, in_maps = _prepare(inputs)
    res = run_bass_kernel_spmd(nc, in_maps, core_ids=list(range(NCORES)))
    return _assemble(res.results)
